# revision 35
# baseline (speedup 1.0000x reference)
"""BasicHypergraphConv on 8 Trainium2 NeuronCores (Bass/Tile, SPMD).

Math: out = scatter_mean_{edges->nodes}( scatter_mean_{nodes->edges}(x[nodes]) @ W.T + b )
The dense linear commutes with the first scatter-mean, so it is applied at the
edge level (5x fewer rows than at the node level).

Distribution (8 cores), v2:
  * Host rebalances BOTH partitions: edges are bin-packed into (core, group)
    bins of 128 edges capped at 15 conn-tiles, nodes into (core, group) bins
    of 128 nodes capped at 3 conn-tiles. This removes the max-over-cores
    padding of the naive contiguous sharding (hop-2 gather 25.7 -> 18.8 MB).
  * hop 1 per core: stream the conn-ordered x rows (bf16, host pre-gather),
    one-hot matmul segment-sum per 128-edge group, 1/cnt_e scale, W^T + b,
    write the [128, D] edge-feature group to ef_loc; AllGather fires per
    2-group slab (5 pipelined AllGathers); a dummy collective at t=0 absorbs
    the CC rendezvous cost.
  * hop 2 per core: SWDGE dma_gather descriptor generation is hoisted into
    hop 1 via prepare_only preps (interleaved with the AllGather triggers on
    the Pool queue using tile_wait_until gates); trigger_dma fires each
    queue's chunks the moment ef_all is complete, so the gather drains at
    full SDMA width instead of being descgen-paced. One-hot matmul
    segment-sum per 128-node group, 1/cnt_n scale, bf16 output block.
Host does index bookkeeping (binning, sort/pad, slab remap) and the final
inverse permutation + fp32 cast.
"""
import numpy as np
import ml_dtypes

import concourse.bass as bass
import concourse.bacc as bacc
import concourse.mybir as mybir
import concourse.tile as tile
from contextlib import ExitStack
from concourse._compat import get_trn_type
from concourse.bass_utils import run_bass_kernel_spmd

NC = 8
P = 128
EG = 10            # edge groups (of 128) per core
SLAB_PLAN = (2, 4, 4)  # edge groups per pipelined AllGather piece; the small
                       # first slab triggers the CC stream (and its one-time
                       # rendezvous barrier) as early as possible
CHUNK_GROUPS = 5   # node groups per hop-2 gather chunk
PROFILE = False
LAST_RESULT = None


def _wrap_idx(idx):
    """dma_gather index layout: [128, n/16] int16; index i lives at
    partition i%16, column i//16, replicated across the 8 groups of 16."""
    n = idx.shape[0]
    a = idx.reshape(n // 16, 16).T.astype(np.int16)
    return np.ascontiguousarray(np.tile(a, (8, 1)))


def _balanced_bins(counts, n_bins, slots_per_bin):
    """Greedy LPT bin-pack: each bin gets exactly `slots_per_bin` ids,
    minimizing the max total count per bin. Returns perm[id] -> new id
    (bin*slots_per_bin + slot) and the per-bin tile bound."""
    order = np.argsort(-counts, kind="stable")
    bin_cnt = np.zeros(n_bins, np.int64)
    bin_fill = np.zeros(n_bins, np.int64)
    perm = np.empty(len(counts), np.int64)
    # iterate heavy ids first; place into least-loaded (by count) open bin
    import heapq
    heap = [(0, 0, b) for b in range(n_bins)]  # (count, fill, bin)
    heapq.heapify(heap)
    for i in order:
        c = counts[i]
        while True:
            cnt, fill, b = heapq.heappop(heap)
            if fill < slots_per_bin:
                break
        perm[i] = b * slots_per_bin + fill
        bin_cnt[b] = cnt + c
        bin_fill[b] = fill + 1
        if fill + 1 < slots_per_bin:
            heapq.heappush(heap, (cnt + c, fill + 1, b))
    tiles = int(max(1, -(-bin_cnt.max() // P)))
    return perm, tiles, bin_cnt


def _per_core_arrays(seg_sorted, gidx_sorted, core, seg_per_core, tiles):
    """Padded per-core gather-index and local-segment arrays (conn axis)."""
    lo = np.searchsorted(seg_sorted, core * seg_per_core)
    hi = np.searchsorted(seg_sorted, (core + 1) * seg_per_core)
    segk = seg_sorted[lo:hi] - core * seg_per_core
    gk = gidx_sorted[lo:hi]
    idx_parts, seg_parts = [], []
    for g, t in enumerate(tiles):
        glo = np.searchsorted(segk, g * P)
        ghi = np.searchsorted(segk, (g + 1) * P)
        m = ghi - glo
        pad = t * P - m
        assert pad >= 0, (g, m, t)
        idx_parts.append(gk[glo:ghi])
        idx_parts.append(np.zeros(pad, np.int64))
        seg_parts.append(segk[glo:ghi] % P)
        seg_parts.append(np.full(pad, -1, np.int64))
    return np.concatenate(idx_parts), np.concatenate(seg_parts)


def _host_prep(x, W, b, nodes, edges):
    n_nodes, d_in = x.shape
    d_out = W.shape[0]
    assert d_in == d_out and d_in % P == 0
    D = d_in
    n_conn = nodes.shape[0]
    nodes = np.asarray(nodes, np.int64)
    edges = np.asarray(edges, np.int64)
    n_edges = int(edges.max()) + 1 if edges.size else 1

    EB = EG * P                                  # edges per core (padded)
    assert NC * EB >= n_edges
    assert sum(SLAB_PLAN) == EG
    slab_base = np.cumsum([0] + list(SLAB_PLAN[:-1])) * P   # edge-row bases
    slab_size = np.asarray(SLAB_PLAN) * P

    # --- balanced permutations ---
    cnt_e_orig = np.bincount(edges, minlength=NC * EB).astype(np.int64)
    eperm, te, _ = _balanced_bins(cnt_e_orig, NC * EG, P)
    t1 = [te] * EG

    # pick NG (node groups/core) so the per-bin conn cap leaves slack
    NG0 = -(-(-(-n_nodes // NC)) // P)
    best = None
    for NG in range(NG0, NG0 + 3):
        cnt_n_orig = np.bincount(nodes, minlength=NC * NG * P).astype(np.int64)
        nperm, tn, _ = _balanced_bins(cnt_n_orig, NC * NG, P)
        if best is None or tn * NG < best[1] * best[0]:
            best = (NG, tn, nperm)
        if tn * NG <= NG0 * 3:
            break
    NG, tn, nperm = best
    NB = NG * P
    t2 = [tn] * NG

    e_new = eperm[edges]
    n_new = nperm[nodes]
    x_bf = np.asarray(x, np.float32).astype(ml_dtypes.bfloat16)

    # hop 1: connections sorted by (new) edge id; gather original x rows
    o1 = np.argsort(e_new, kind="stable")
    e1 = e_new[o1]
    xrow1 = nodes[o1]                            # original node ids -> x rows
    # hop 2: connections sorted by (new) node id; edge ids remapped to the
    # slab-wise AllGather table layout
    o2 = np.argsort(n_new, kind="stable")
    n2, e2 = n_new[o2], e_new[o2]
    r2, loc2 = e2 // EB, e2 % EB
    s2i = np.searchsorted(slab_base, loc2, side="right") - 1
    e2m = (NC * slab_base[s2i] + r2 * slab_size[s2i]
           + (loc2 - slab_base[s2i]))
    assert NC * EB <= 32768 - 1

    cnt_e = np.bincount(e_new, minlength=NC * EB).astype(np.float32)
    cnt_n = np.bincount(n_new, minlength=NC * NB).astype(np.float32)
    recip_e = 1.0 / np.maximum(cnt_e, 1.0)
    recip_n = 1.0 / np.maximum(cnt_n, 1.0)

    # weight in lhs-chunk layout: wt4[p, c*D+o] = W[o, 128c+p]
    nchunk = D // P
    wt4 = (
        np.asarray(W, np.float32).T.reshape(nchunk, P, D).transpose(1, 0, 2)
        .reshape(P, nchunk * D).astype(ml_dtypes.bfloat16)
    )
    bias = np.broadcast_to(np.asarray(b, np.float32), (P, D)).copy()

    T1sum, T2sum = sum(t1), sum(t2)
    in_maps = []
    for k in range(NC):
        i1raw, s1 = _per_core_arrays(e1, xrow1, k, EB, t1)
        i2, s2 = _per_core_arrays(n2, e2m, k, NB, t2)
        xg = x_bf[i1raw].reshape(T1sum, P, D).transpose(1, 0, 2).reshape(P, T1sum * D)
        in_maps.append({
            "xg": np.ascontiguousarray(xg),
            "wt4": wt4,
            "bias": bias,
            "dum": np.zeros((16, 4), np.float32),
            "s1": np.ascontiguousarray(s1.reshape(T1sum, P).T.astype(np.float32)),
            "re": np.ascontiguousarray(
                recip_e[k * EB:(k + 1) * EB].reshape(EG, P).T),
            "g2i": _wrap_idx(i2),
            "s2": np.ascontiguousarray(s2.reshape(T2sum, P).T.astype(np.float32)),
            "rn": np.ascontiguousarray(
                recip_n[k * NB:(k + 1) * NB].reshape(NG, P).T),
        })
    dims = dict(D=D, EB=EB, NB=NB, NG=NG, t1=tuple(t1), t2=tuple(t2))
    return dims, in_maps, n_nodes, nperm


def _build_program(dims):
    D, EB, NB = dims["D"], dims["EB"], dims["NB"]
    t1, t2 = list(dims["t1"]), list(dims["t2"])
    NG = dims["NG"]
    T1sum, T2sum = sum(t1), sum(t2)
    nchunk = D // P
    slab_end = np.cumsum(SLAB_PLAN)          # AG after these group counts
    slab_base = np.cumsum([0] + list(SLAB_PLAN[:-1])) * P
    slab_size = np.asarray(SLAB_PLAN) * P
    dt = mybir.dt

    # hop-2 chunks: CHUNK_GROUPS node groups each
    chunks = []
    g0 = 0
    while g0 < NG:
        ng = min(CHUNK_GROUPS, NG - g0)
        chunks.append((g0, ng, sum(t2[g0:g0 + ng])))
        g0 += ng
    nch = len(chunks)
    chmax = max(max(c[2] for c in chunks), max(t1))

    nc = bacc.Bacc(get_trn_type() or "TRN2", target_bir_lowering=False,
                   debug=False, num_devices=NC, num_swdge_queues=4,
                   dynamic_dma_scratch_size=32768)
    xg = nc.dram_tensor("xg", [P, T1sum * D], dt.bfloat16, kind="ExternalInput")
    wt4 = nc.dram_tensor("wt4", [P, nchunk * D], dt.bfloat16, kind="ExternalInput")
    bias = nc.dram_tensor("bias", [P, D], dt.float32, kind="ExternalInput")
    dum = nc.dram_tensor("dum", [16, 4], dt.float32, kind="ExternalInput")
    s1 = nc.dram_tensor("s1", [P, T1sum], dt.float32, kind="ExternalInput")
    re_ = nc.dram_tensor("re", [P, EG], dt.float32, kind="ExternalInput")
    g2i = nc.dram_tensor("g2i", [P, T2sum * 8], dt.int16, kind="ExternalInput")
    s2 = nc.dram_tensor("s2", [P, T2sum], dt.float32, kind="ExternalInput")
    rn = nc.dram_tensor("rn", [P, NG], dt.float32, kind="ExternalInput")
    out = nc.dram_tensor("out", [NB, D], dt.bfloat16, kind="ExternalOutput")

    with tile.TileContext(nc) as tc, ExitStack() as ctx:
        res = ctx.enter_context(tc.tile_pool(name="res", bufs=1))
        g1pool = ctx.enter_context(tc.tile_pool(name="g1", bufs=2))
        g2pool = ctx.enter_context(tc.tile_pool(name="g2", bufs=6))
        spool = ctx.enter_context(tc.tile_pool(name="oneh", bufs=3))
        epool = ctx.enter_context(tc.tile_pool(name="ef", bufs=3))
        opool = ctx.enter_context(tc.tile_pool(name="osb", bufs=3))
        pseg = ctx.enter_context(tc.tile_pool(name="pseg", bufs=2, space="PSUM"))
        pw = ctx.enter_context(tc.tile_pool(name="pw", bufs=2, space="PSUM"))
        pt = ctx.enter_context(tc.tile_pool(name="pt", bufs=2, space="PSUM"))
        dram = ctx.enter_context(tc.tile_pool(name="dram", bufs=1, space="DRAM"))

        # ---- resident data
        g2i_sb = res.tile([P, T2sum * 8], dt.int16)
        nc.sync.dma_start(g2i_sb[:], g2i[:])
        wt_sb = res.tile([P, nchunk * D], dt.bfloat16)
        nc.sync.dma_start(wt_sb[:], wt4[:])
        bias_sb = res.tile([P, D], dt.float32)
        nc.sync.dma_start(bias_sb[:], bias[:])
        s1_sb = res.tile([P, T1sum], dt.float32)
        nc.sync.dma_start(s1_sb[:], s1[:])
        re_sb = res.tile([P, EG], dt.float32)
        nc.sync.dma_start(re_sb[:], re_[:])
        s2_sb = res.tile([P, T2sum], dt.float32)
        nc.sync.dma_start(s2_sb[:], s2[:])
        rn_sb = res.tile([P, NG], dt.float32)
        nc.sync.dma_start(rn_sb[:], rn[:])

        # the CC stream's one-time rendezvous barrier is absorbed by the
        # first (small, early) AllGather slab — no dummy collective needed
        dum_sb = res.tile([16, 4], dt.float32)
        nc.sync.dma_start(dum_sb[:], dum[:])
        cc_names = []

        # iota_wide[p, t*128+j] = j  (for batched one-hot builds)
        ioww_i = res.tile([P, chmax * P], dt.int32)
        nc.gpsimd.iota(ioww_i[:], pattern=[[0, chmax], [1, P]], base=0,
                       channel_multiplier=0)
        iota_w = res.tile([P, chmax * P], dt.float32)
        nc.vector.tensor_copy(iota_w[:], ioww_i[:])
        diag_i = res.tile([P, 1], dt.int32)
        nc.gpsimd.iota(diag_i[:], pattern=[[0, 1]], base=0, channel_multiplier=1)
        diag_f = res.tile([P, 1], dt.float32)
        nc.vector.tensor_copy(diag_f[:], diag_i[:])
        ident = res.tile([P, P], dt.bfloat16)
        nc.vector.tensor_scalar(ident[:], iota_w[:, :P], diag_f[:], None,
                                mybir.AluOpType.is_equal)

        ef_loc = dram.tile([EB, D], dt.bfloat16)
        ef_all = nc.dram_tensor("ef_all_sh", [NC * EB, D], dt.bfloat16,
                                addr_space="Shared")

        # ---- hop 1: conn-ordered x stream -> edge means -> @W.T + b -> ef_loc
        tbase = 0
        for g in range(EG):
            ct = t1[g]
            gb = g1pool.tile([P, chmax * D], dt.bfloat16, tag="g1buf")
            nc.sync.dma_start(gb[:, :ct * D], xg[:, tbase * D:(tbase + ct) * D])
            sc = spool.tile([P, chmax * P], dt.bfloat16, tag="oh")
            nc.vector.tensor_tensor(
                sc[:, :ct * P].rearrange("p (c q) -> p c q", q=P),
                iota_w[:, :ct * P].rearrange("p (c q) -> p c q", q=P),
                s1_sb[:, tbase:tbase + ct].broadcast_to((P, ct, P)),
                mybir.AluOpType.is_equal)
            psum = pseg.tile([P, D], dt.float32, tag="pseg")
            for t in range(ct):
                nc.tensor.matmul(psum[:], sc[:, t * P:(t + 1) * P],
                                 gb[:, t * D:(t + 1) * D],
                                 start=(t == 0), stop=(t == ct - 1))
            ef_sb = epool.tile([P, D], dt.bfloat16, tag="efm")
            nc.vector.tensor_scalar(ef_sb[:], psum[:], re_sb[:, g:g + 1],
                                    None, mybir.AluOpType.mult)
            pw_t = pw.tile([P, D], dt.float32, tag="pw")
            for c in range(nchunk):
                ptt = pt.tile([P, P], dt.bfloat16, tag="pt")
                nc.tensor.transpose(ptt[:], ef_sb[:, c * P:(c + 1) * P], ident[:])
                efT = epool.tile([P, P], dt.bfloat16, tag="efT")
                nc.scalar.copy(efT[:], ptt[:])
                nc.tensor.matmul(pw_t[:], efT[:], wt_sb[:, c * D:(c + 1) * D],
                                 start=(c == 0), stop=(c == nchunk - 1))
            efp = epool.tile([P, D], dt.bfloat16, tag="efp")
            nc.vector.tensor_add(efp[:], pw_t[:], bias_sb[:])
            nc.sync.dma_start(ef_loc[g * P:(g + 1) * P, :], efp[:])
            tbase += ct
            # fire this slab's AllGather as soon as its groups are written
            if (g + 1) in slab_end:
                s = int(np.searchsorted(slab_end, g + 1))
                b, sz = int(slab_base[s]), int(slab_size[s])
                cc_names.append(nc.gpsimd.collective_compute(
                    "AllGather", mybir.AluOpType.bypass,
                    ins=[ef_loc[b:b + sz, :]],
                    outs=[ef_all[NC * b:NC * (b + sz), :]],
                    replica_groups=[list(range(NC))]).ins.name)

        # ---- hop 2 descriptor preps (descgen overlaps hop 1 / AllGathers).
        # The framework pins the ef_all RAW dep on the prep, which would
        # serialize descgen behind the collectives — strip those edges from
        # the preps (descriptors only read the index SBUF) and instead make
        # each trigger claim a write on ef_all, so the trigger orders after
        # the AllGathers via WAW while descgen runs during hop 1.
        # 1:1 prep->trigger (count=None right before the next prep on the
        # same queue) keeps per-chunk DMA-completion accounting exact.
        gsem = [nc.alloc_semaphore(f"gsem{q}") for q in range(4)]
        gb2 = []
        prep_gate_us = [3, 16, 29, 42]
        N_EARLY = 4          # bounded by the SWDGE descriptor-ring capacity
        late_gate_us = {ci: 95 + 18 * (ci - N_EARLY) for ci in range(N_EARLY, nch)}
        tbase_of = []
        tbase = 0
        for ci, (g0, ng, ctiles) in enumerate(chunks):
            tbase_of.append(tbase)
            tbase += ctiles

        # Inline gathers: descgen waits for the AllGathers (the framework
        # pins the ef_all RAW dep on the gather op) and paces hop 2, but the
        # 4 queues drain concurrently behind it.
        for ci, (g0, ng, ctiles) in enumerate(chunks):
            gb = g2pool.tile([P, chmax * D], dt.bfloat16, tag="g2buf")
            gb2.append(gb)
            nc.gpsimd.dma_gather(
                gb[:, :ctiles * D].rearrange("p (c q) -> p c q", q=D),
                ef_all[:],
                g2i_sb[:, tbase_of[ci] * 8:(tbase_of[ci] + ctiles) * 8],
                ctiles * P, ctiles * P, D,
                single_packet=False, queue_num=ci % 4)

        # ---- hop 2 compute: ef rows -> node means -> out
        tbase = 0
        for ci, (g0, ng, ctiles) in enumerate(chunks):
            gb = gb2[ci]
            sc = spool.tile([P, chmax * P], dt.bfloat16, tag="oh")
            nc.vector.tensor_tensor(
                sc[:, :ctiles * P].rearrange("p (c q) -> p c q", q=P),
                iota_w[:, :ctiles * P].rearrange("p (c q) -> p c q", q=P),
                s2_sb[:, tbase:tbase + ctiles].broadcast_to((P, ctiles, P)),
                mybir.AluOpType.is_equal)
            toff = 0
            for g in range(g0, g0 + ng):
                psum = pseg.tile([P, D], dt.float32, tag="pseg")
                for t in range(t2[g]):
                    tt = toff + t
                    nc.tensor.matmul(psum[:], sc[:, tt * P:(tt + 1) * P],
                                     gb[:, tt * D:(tt + 1) * D],
                                     start=(t == 0), stop=(t == t2[g] - 1))
                o_sb = opool.tile([P, D], dt.bfloat16, tag="osb")
                nc.vector.tensor_scalar(o_sb[:], psum[:], rn_sb[:, g:g + 1],
                                        None, mybir.AluOpType.mult)
                nc.sync.dma_start(out[g * P:(g + 1) * P, :], o_sb[:])
                toff += t2[g]
            tbase += ctiles

    nc.compile()
    return nc


_PROGRAM_CACHE = {}


def kernel(**inputs):
    x = np.asarray(inputs["x"], np.float32)
    W = np.asarray(inputs["W"], np.float32)
    b = np.asarray(inputs["b"], np.float32)
    nodes = np.asarray(inputs["nodes"])
    edges = np.asarray(inputs["edges"])

    dims, in_maps, n_nodes, nperm = _host_prep(x, W, b, nodes, edges)
    key = (dims["D"], dims["EB"], dims["NB"], SLAB_PLAN, dims["t1"], dims["t2"])
    nc = _PROGRAM_CACHE.get(key)
    if nc is None:
        nc = _build_program(dims)
        _PROGRAM_CACHE[key] = nc

    global LAST_RESULT
    res = run_bass_kernel_spmd(nc, in_maps, list(range(NC)), trace=PROFILE)
    LAST_RESULT = res
    out_new = np.concatenate(
        [np.asarray(res.results[k]["out"]) for k in range(NC)], axis=0)
    out = out_new[nperm[np.arange(n_nodes)]].astype(np.float32)
    return np.ascontiguousarray(out)


# revision 37
# speedup vs baseline: 1.0662x; 1.0662x over previous
"""BasicHypergraphConv on 8 Trainium2 NeuronCores (Bass/Tile, SPMD).

Math: out = scatter_mean_{edges->nodes}( scatter_mean_{nodes->edges}(x[nodes]) @ W.T + b )
The dense linear commutes with the first scatter-mean, so it is applied at the
edge level (5x fewer rows than at the node level).

Distribution (8 cores), v2:
  * Host rebalances BOTH partitions: edges are bin-packed into (core, group)
    bins of 128 edges capped at 15 conn-tiles, nodes into (core, group) bins
    of 128 nodes capped at 3 conn-tiles. This removes the max-over-cores
    padding of the naive contiguous sharding (hop-2 gather 25.7 -> 18.8 MB).
  * hop 1 per core: stream the conn-ordered x rows (bf16, host pre-gather),
    one-hot matmul segment-sum per 128-edge group, 1/cnt_e scale, W^T + b,
    write the [128, D] edge-feature group to ef_loc; AllGather fires per
    2-group slab (5 pipelined AllGathers); a dummy collective at t=0 absorbs
    the CC rendezvous cost.
  * hop 2 per core: SWDGE dma_gather descriptor generation is hoisted into
    hop 1 via prepare_only preps (interleaved with the AllGather triggers on
    the Pool queue using tile_wait_until gates); trigger_dma fires each
    queue's chunks the moment ef_all is complete, so the gather drains at
    full SDMA width instead of being descgen-paced. One-hot matmul
    segment-sum per 128-node group, 1/cnt_n scale, bf16 output block.
Host does index bookkeeping (binning, sort/pad, slab remap) and the final
inverse permutation + fp32 cast.
"""
import numpy as np
import ml_dtypes

import concourse.bass as bass
import concourse.bacc as bacc
import concourse.mybir as mybir
import concourse.tile as tile
from contextlib import ExitStack
from concourse._compat import get_trn_type
from concourse.bass_utils import run_bass_kernel_spmd

NC = 8
P = 128
EG = 10            # edge groups (of 128) per core
SLAB_PLAN = (5, 5)     # edge groups per pipelined AllGather piece. Two ops:
                       # each CC op costs ~15-18us fixed + ~15us gap, and a
                       # dummy collective at t=0 absorbs the rendezvous
                       # barrier (measured: without it the barrier runs 84us)
CHUNK_GROUPS = 5   # node groups per hop-2 gather chunk
PROFILE = False
LAST_RESULT = None


def _wrap_idx(idx):
    """dma_gather index layout: [128, n/16] int16; index i lives at
    partition i%16, column i//16, replicated across the 8 groups of 16."""
    n = idx.shape[0]
    a = idx.reshape(n // 16, 16).T.astype(np.int16)
    return np.ascontiguousarray(np.tile(a, (8, 1)))


def _balanced_bins(counts, n_bins, slots_per_bin):
    """Greedy LPT bin-pack: each bin gets exactly `slots_per_bin` ids,
    minimizing the max total count per bin. Returns perm[id] -> new id
    (bin*slots_per_bin + slot) and the per-bin tile bound."""
    order = np.argsort(-counts, kind="stable")
    bin_cnt = np.zeros(n_bins, np.int64)
    bin_fill = np.zeros(n_bins, np.int64)
    perm = np.empty(len(counts), np.int64)
    # iterate heavy ids first; place into least-loaded (by count) open bin
    import heapq
    heap = [(0, 0, b) for b in range(n_bins)]  # (count, fill, bin)
    heapq.heapify(heap)
    for i in order:
        c = counts[i]
        while True:
            cnt, fill, b = heapq.heappop(heap)
            if fill < slots_per_bin:
                break
        perm[i] = b * slots_per_bin + fill
        bin_cnt[b] = cnt + c
        bin_fill[b] = fill + 1
        if fill + 1 < slots_per_bin:
            heapq.heappush(heap, (cnt + c, fill + 1, b))
    tiles = int(max(1, -(-bin_cnt.max() // P)))
    return perm, tiles, bin_cnt


def _per_core_arrays(seg_sorted, gidx_sorted, core, seg_per_core, tiles):
    """Padded per-core gather-index and local-segment arrays (conn axis)."""
    lo = np.searchsorted(seg_sorted, core * seg_per_core)
    hi = np.searchsorted(seg_sorted, (core + 1) * seg_per_core)
    segk = seg_sorted[lo:hi] - core * seg_per_core
    gk = gidx_sorted[lo:hi]
    idx_parts, seg_parts = [], []
    for g, t in enumerate(tiles):
        glo = np.searchsorted(segk, g * P)
        ghi = np.searchsorted(segk, (g + 1) * P)
        m = ghi - glo
        pad = t * P - m
        assert pad >= 0, (g, m, t)
        idx_parts.append(gk[glo:ghi])
        idx_parts.append(np.zeros(pad, np.int64))
        seg_parts.append(segk[glo:ghi] % P)
        seg_parts.append(np.full(pad, -1, np.int64))
    return np.concatenate(idx_parts), np.concatenate(seg_parts)


def _host_prep(x, W, b, nodes, edges):
    n_nodes, d_in = x.shape
    d_out = W.shape[0]
    assert d_in == d_out and d_in % P == 0
    D = d_in
    n_conn = nodes.shape[0]
    nodes = np.asarray(nodes, np.int64)
    edges = np.asarray(edges, np.int64)
    n_edges = int(edges.max()) + 1 if edges.size else 1

    EB = EG * P                                  # edges per core (padded)
    assert NC * EB >= n_edges
    assert sum(SLAB_PLAN) == EG
    slab_base = np.cumsum([0] + list(SLAB_PLAN[:-1])) * P   # edge-row bases
    slab_size = np.asarray(SLAB_PLAN) * P

    # --- balanced permutations ---
    cnt_e_orig = np.bincount(edges, minlength=NC * EB).astype(np.int64)
    eperm, te, _ = _balanced_bins(cnt_e_orig, NC * EG, P)
    t1 = [te] * EG

    # pick NG (node groups/core) so the per-bin conn cap leaves slack
    NG0 = -(-(-(-n_nodes // NC)) // P)
    best = None
    for NG in range(NG0, NG0 + 3):
        cnt_n_orig = np.bincount(nodes, minlength=NC * NG * P).astype(np.int64)
        nperm, tn, _ = _balanced_bins(cnt_n_orig, NC * NG, P)
        if best is None or tn * NG < best[1] * best[0]:
            best = (NG, tn, nperm)
        if tn * NG <= NG0 * 3:
            break
    NG, tn, nperm = best
    NB = NG * P
    t2 = [tn] * NG

    e_new = eperm[edges]
    n_new = nperm[nodes]
    x_bf = np.asarray(x, np.float32).astype(ml_dtypes.bfloat16)

    # hop 1: connections sorted by (new) edge id; gather original x rows
    o1 = np.argsort(e_new, kind="stable")
    e1 = e_new[o1]
    xrow1 = nodes[o1]                            # original node ids -> x rows
    # hop 2: connections sorted by (new) node id; edge ids remapped to the
    # slab-wise AllGather table layout
    o2 = np.argsort(n_new, kind="stable")
    n2, e2 = n_new[o2], e_new[o2]
    r2, loc2 = e2 // EB, e2 % EB
    s2i = np.searchsorted(slab_base, loc2, side="right") - 1
    e2m = (NC * slab_base[s2i] + r2 * slab_size[s2i]
           + (loc2 - slab_base[s2i]))
    assert NC * EB <= 32768 - 1

    cnt_e = np.bincount(e_new, minlength=NC * EB).astype(np.float32)
    cnt_n = np.bincount(n_new, minlength=NC * NB).astype(np.float32)
    recip_e = 1.0 / np.maximum(cnt_e, 1.0)
    recip_n = 1.0 / np.maximum(cnt_n, 1.0)

    # weight in lhs-chunk layout: wt4[p, c*D+o] = W[o, 128c+p]
    nchunk = D // P
    wt4 = (
        np.asarray(W, np.float32).T.reshape(nchunk, P, D).transpose(1, 0, 2)
        .reshape(P, nchunk * D).astype(ml_dtypes.bfloat16)
    )
    bias = np.broadcast_to(np.asarray(b, np.float32), (P, D)).copy()

    T1sum, T2sum = sum(t1), sum(t2)
    in_maps = []
    for k in range(NC):
        i1raw, s1 = _per_core_arrays(e1, xrow1, k, EB, t1)
        i2, s2 = _per_core_arrays(n2, e2m, k, NB, t2)
        xg = x_bf[i1raw].reshape(T1sum, P, D).transpose(1, 0, 2).reshape(P, T1sum * D)
        in_maps.append({
            "xg": np.ascontiguousarray(xg),
            "wt4": wt4,
            "bias": bias,
            "dum": np.zeros((16, 4), np.float32),
            "s1": np.ascontiguousarray(s1.reshape(T1sum, P).T.astype(np.float32)),
            "re": np.ascontiguousarray(
                recip_e[k * EB:(k + 1) * EB].reshape(EG, P).T),
            "g2i": _wrap_idx(i2),
            "s2": np.ascontiguousarray(s2.reshape(T2sum, P).T.astype(np.float32)),
            "rn": np.ascontiguousarray(
                recip_n[k * NB:(k + 1) * NB].reshape(NG, P).T),
        })
    dims = dict(D=D, EB=EB, NB=NB, NG=NG, t1=tuple(t1), t2=tuple(t2))
    return dims, in_maps, n_nodes, nperm


def _build_program(dims):
    D, EB, NB = dims["D"], dims["EB"], dims["NB"]
    t1, t2 = list(dims["t1"]), list(dims["t2"])
    NG = dims["NG"]
    T1sum, T2sum = sum(t1), sum(t2)
    nchunk = D // P
    slab_end = np.cumsum(SLAB_PLAN)          # AG after these group counts
    slab_base = np.cumsum([0] + list(SLAB_PLAN[:-1])) * P
    slab_size = np.asarray(SLAB_PLAN) * P
    dt = mybir.dt

    # hop-2 chunks: CHUNK_GROUPS node groups each
    chunks = []
    g0 = 0
    while g0 < NG:
        ng = min(CHUNK_GROUPS, NG - g0)
        chunks.append((g0, ng, sum(t2[g0:g0 + ng])))
        g0 += ng
    nch = len(chunks)
    chmax = max(max(c[2] for c in chunks), max(t1))

    nc = bacc.Bacc(get_trn_type() or "TRN2", target_bir_lowering=False,
                   debug=False, num_devices=NC, num_swdge_queues=4,
                   dynamic_dma_scratch_size=32768)
    xg = nc.dram_tensor("xg", [P, T1sum * D], dt.bfloat16, kind="ExternalInput")
    wt4 = nc.dram_tensor("wt4", [P, nchunk * D], dt.bfloat16, kind="ExternalInput")
    bias = nc.dram_tensor("bias", [P, D], dt.float32, kind="ExternalInput")
    dum = nc.dram_tensor("dum", [16, 4], dt.float32, kind="ExternalInput")
    s1 = nc.dram_tensor("s1", [P, T1sum], dt.float32, kind="ExternalInput")
    re_ = nc.dram_tensor("re", [P, EG], dt.float32, kind="ExternalInput")
    g2i = nc.dram_tensor("g2i", [P, T2sum * 8], dt.int16, kind="ExternalInput")
    s2 = nc.dram_tensor("s2", [P, T2sum], dt.float32, kind="ExternalInput")
    rn = nc.dram_tensor("rn", [P, NG], dt.float32, kind="ExternalInput")
    out = nc.dram_tensor("out", [NB, D], dt.bfloat16, kind="ExternalOutput")

    with tile.TileContext(nc) as tc, ExitStack() as ctx:
        res = ctx.enter_context(tc.tile_pool(name="res", bufs=1))
        g1pool = ctx.enter_context(tc.tile_pool(name="g1", bufs=2))
        g2pool = ctx.enter_context(tc.tile_pool(name="g2", bufs=6))
        spool = ctx.enter_context(tc.tile_pool(name="oneh", bufs=3))
        epool = ctx.enter_context(tc.tile_pool(name="ef", bufs=3))
        opool = ctx.enter_context(tc.tile_pool(name="osb", bufs=3))
        pseg = ctx.enter_context(tc.tile_pool(name="pseg", bufs=2, space="PSUM"))
        pw = ctx.enter_context(tc.tile_pool(name="pw", bufs=2, space="PSUM"))
        pt = ctx.enter_context(tc.tile_pool(name="pt", bufs=2, space="PSUM"))
        dram = ctx.enter_context(tc.tile_pool(name="dram", bufs=1, space="DRAM"))

        # ---- resident data
        g2i_sb = res.tile([P, T2sum * 8], dt.int16)
        nc.sync.dma_start(g2i_sb[:], g2i[:])
        wt_sb = res.tile([P, nchunk * D], dt.bfloat16)
        nc.sync.dma_start(wt_sb[:], wt4[:])
        bias_sb = res.tile([P, D], dt.float32)
        nc.sync.dma_start(bias_sb[:], bias[:])
        s1_sb = res.tile([P, T1sum], dt.float32)
        nc.sync.dma_start(s1_sb[:], s1[:])
        re_sb = res.tile([P, EG], dt.float32)
        nc.sync.dma_start(re_sb[:], re_[:])
        s2_sb = res.tile([P, T2sum], dt.float32)
        nc.sync.dma_start(s2_sb[:], s2[:])
        rn_sb = res.tile([P, NG], dt.float32)
        nc.sync.dma_start(rn_sb[:], rn[:])

        # dummy collective at t=0 absorbs the CC stream rendezvous barrier
        dum_out = nc.dram_tensor("dum_out_sh", [16 * NC, 4], dt.float32,
                                 addr_space="Shared")
        dum_sb = res.tile([16, 4], dt.float32)
        nc.sync.dma_start(dum_sb[:], dum[:])
        dum_dr = dram.tile([16, 4], dt.float32)
        nc.sync.dma_start(dum_dr[:], dum_sb[:])
        cc_names = []
        cc_names.append(nc.gpsimd.collective_compute(
            "AllGather", mybir.AluOpType.bypass,
            ins=[dum_dr[:]], outs=[dum_out[:]],
            replica_groups=[list(range(NC))]).ins.name)

        # iota_wide[p, t*128+j] = j  (for batched one-hot builds)
        ioww_i = res.tile([P, chmax * P], dt.int32)
        nc.gpsimd.iota(ioww_i[:], pattern=[[0, chmax], [1, P]], base=0,
                       channel_multiplier=0)
        iota_w = res.tile([P, chmax * P], dt.float32)
        nc.vector.tensor_copy(iota_w[:], ioww_i[:])
        diag_i = res.tile([P, 1], dt.int32)
        nc.gpsimd.iota(diag_i[:], pattern=[[0, 1]], base=0, channel_multiplier=1)
        diag_f = res.tile([P, 1], dt.float32)
        nc.vector.tensor_copy(diag_f[:], diag_i[:])
        ident = res.tile([P, P], dt.bfloat16)
        nc.vector.tensor_scalar(ident[:], iota_w[:, :P], diag_f[:], None,
                                mybir.AluOpType.is_equal)

        ef_loc = dram.tile([EB, D], dt.bfloat16)
        ef_all = nc.dram_tensor("ef_all_sh", [NC * EB, D], dt.bfloat16,
                                addr_space="Shared")

        # ---- hop 1: conn-ordered x stream -> edge means -> @W.T + b -> ef_loc
        tbase = 0
        for g in range(EG):
            ct = t1[g]
            gb = g1pool.tile([P, chmax * D], dt.bfloat16, tag="g1buf")
            nc.sync.dma_start(gb[:, :ct * D], xg[:, tbase * D:(tbase + ct) * D])
            sc = spool.tile([P, chmax * P], dt.bfloat16, tag="oh")
            nc.vector.tensor_tensor(
                sc[:, :ct * P].rearrange("p (c q) -> p c q", q=P),
                iota_w[:, :ct * P].rearrange("p (c q) -> p c q", q=P),
                s1_sb[:, tbase:tbase + ct].broadcast_to((P, ct, P)),
                mybir.AluOpType.is_equal)
            psum = pseg.tile([P, D], dt.float32, tag="pseg")
            for t in range(ct):
                nc.tensor.matmul(psum[:], sc[:, t * P:(t + 1) * P],
                                 gb[:, t * D:(t + 1) * D],
                                 start=(t == 0), stop=(t == ct - 1))
            ef_sb = epool.tile([P, D], dt.bfloat16, tag="efm")
            nc.vector.tensor_scalar(ef_sb[:], psum[:], re_sb[:, g:g + 1],
                                    None, mybir.AluOpType.mult)
            pw_t = pw.tile([P, D], dt.float32, tag="pw")
            for c in range(nchunk):
                ptt = pt.tile([P, P], dt.bfloat16, tag="pt")
                nc.tensor.transpose(ptt[:], ef_sb[:, c * P:(c + 1) * P], ident[:])
                efT = epool.tile([P, P], dt.bfloat16, tag="efT")
                nc.scalar.copy(efT[:], ptt[:])
                nc.tensor.matmul(pw_t[:], efT[:], wt_sb[:, c * D:(c + 1) * D],
                                 start=(c == 0), stop=(c == nchunk - 1))
            efp = epool.tile([P, D], dt.bfloat16, tag="efp")
            nc.vector.tensor_add(efp[:], pw_t[:], bias_sb[:])
            nc.sync.dma_start(ef_loc[g * P:(g + 1) * P, :], efp[:])
            tbase += ct
            # fire this slab's AllGather as soon as its groups are written
            if (g + 1) in slab_end:
                s = int(np.searchsorted(slab_end, g + 1))
                b, sz = int(slab_base[s]), int(slab_size[s])
                cc_names.append(nc.gpsimd.collective_compute(
                    "AllGather", mybir.AluOpType.bypass,
                    ins=[ef_loc[b:b + sz, :]],
                    outs=[ef_all[NC * b:NC * (b + sz), :]],
                    replica_groups=[list(range(NC))]).ins.name)

        # ---- hop 2 descriptor preps (descgen overlaps hop 1 / AllGathers).
        # The framework pins the ef_all RAW dep on the prep, which would
        # serialize descgen behind the collectives — strip those edges from
        # the preps (descriptors only read the index SBUF) and instead make
        # each trigger claim a write on ef_all, so the trigger orders after
        # the AllGathers via WAW while descgen runs during hop 1.
        # 1:1 prep->trigger (count=None right before the next prep on the
        # same queue) keeps per-chunk DMA-completion accounting exact.
        gsem = [nc.alloc_semaphore(f"gsem{q}") for q in range(4)]
        gb2 = []
        prep_gate_us = [3, 16, 29, 42]
        N_EARLY = 4          # bounded by the SWDGE descriptor-ring capacity
        late_gate_us = {ci: 95 + 18 * (ci - N_EARLY) for ci in range(N_EARLY, nch)}
        tbase_of = []
        tbase = 0
        for ci, (g0, ng, ctiles) in enumerate(chunks):
            tbase_of.append(tbase)
            tbase += ctiles

        # Inline gathers: descgen waits for the AllGathers (the framework
        # pins the ef_all RAW dep on the gather op) and paces hop 2, but the
        # 4 queues drain concurrently behind it.
        for ci, (g0, ng, ctiles) in enumerate(chunks):
            gb = g2pool.tile([P, chmax * D], dt.bfloat16, tag="g2buf")
            gb2.append(gb)
            nc.gpsimd.dma_gather(
                gb[:, :ctiles * D].rearrange("p (c q) -> p c q", q=D),
                ef_all[:],
                g2i_sb[:, tbase_of[ci] * 8:(tbase_of[ci] + ctiles) * 8],
                ctiles * P, ctiles * P, D,
                single_packet=False, queue_num=ci % 4)

        # ---- hop 2 compute: ef rows -> node means -> out
        tbase = 0
        for ci, (g0, ng, ctiles) in enumerate(chunks):
            gb = gb2[ci]
            sc = spool.tile([P, chmax * P], dt.bfloat16, tag="oh")
            nc.vector.tensor_tensor(
                sc[:, :ctiles * P].rearrange("p (c q) -> p c q", q=P),
                iota_w[:, :ctiles * P].rearrange("p (c q) -> p c q", q=P),
                s2_sb[:, tbase:tbase + ctiles].broadcast_to((P, ctiles, P)),
                mybir.AluOpType.is_equal)
            toff = 0
            for g in range(g0, g0 + ng):
                psum = pseg.tile([P, D], dt.float32, tag="pseg")
                for t in range(t2[g]):
                    tt = toff + t
                    nc.tensor.matmul(psum[:], sc[:, tt * P:(tt + 1) * P],
                                     gb[:, tt * D:(tt + 1) * D],
                                     start=(t == 0), stop=(t == t2[g] - 1))
                o_sb = opool.tile([P, D], dt.bfloat16, tag="osb")
                nc.vector.tensor_scalar(o_sb[:], psum[:], rn_sb[:, g:g + 1],
                                        None, mybir.AluOpType.mult)
                nc.sync.dma_start(out[g * P:(g + 1) * P, :], o_sb[:])
                toff += t2[g]
            tbase += ctiles

    nc.compile()
    return nc


_PROGRAM_CACHE = {}


def kernel(**inputs):
    x = np.asarray(inputs["x"], np.float32)
    W = np.asarray(inputs["W"], np.float32)
    b = np.asarray(inputs["b"], np.float32)
    nodes = np.asarray(inputs["nodes"])
    edges = np.asarray(inputs["edges"])

    dims, in_maps, n_nodes, nperm = _host_prep(x, W, b, nodes, edges)
    key = (dims["D"], dims["EB"], dims["NB"], SLAB_PLAN, dims["t1"], dims["t2"])
    nc = _PROGRAM_CACHE.get(key)
    if nc is None:
        nc = _build_program(dims)
        _PROGRAM_CACHE[key] = nc

    global LAST_RESULT
    res = run_bass_kernel_spmd(nc, in_maps, list(range(NC)), trace=PROFILE)
    LAST_RESULT = res
    out_new = np.concatenate(
        [np.asarray(res.results[k]["out"]) for k in range(NC)], axis=0)
    out = out_new[nperm[np.arange(n_nodes)]].astype(np.float32)
    return np.ascontiguousarray(out)
